# revision 10
# baseline (speedup 1.0000x reference)
"""BitLinear158 Trainium2 kernel — fp8 DoubleRow with partial hi/lo correction.

Reference computation:
    gamma = mean(|W|)
    Wq    = clip(round(W / (gamma + 1e-5)), -1, 1)      # ternary {-1, 0, +1}
    out   = x @ Wq.T + b                                # x: [8, 4096, 2048]

Sharding: data-parallel over the batch dim (8 batches -> 8 cores). Each core
gets x[i] (host-transposed to k-major), the full W (host-transposed) and b.
gamma is computed redundantly per-core -- measured cross-core collective
latency/skew (~80us) far exceeds the 45us it would save.

Math: Wq is ternary so it is EXACT in fp8e4 (e4m3). The fp8 DoubleRow matmul
contracts K=256 per instruction at the same per-instruction cost as a bf16
K=128 matmul (measured ~216ns at 512 free rows) -> 2x FLOP rate. Activations
split x = hi + lo with hi = fp8(x), lo = fp8(x - hi); hi covers all 16
k-tiles, lo corrects k-tiles 8..15 (L=4 of 8 k-pairs). Output L2 rel error
= 2.35e-2 * sqrt(1 - L/8) -> 1.66e-2 measured on HW (gate 2e-2).

Device pipeline per core:
  pass 1: stream WT (16 MiB, DMA-bound ~50us). During the stream: gamma
          partials via DVE reduce_sum(|.|) on 10 tiles and ACT Abs+accum on
          6; the ACT path's |W| output is KEPT in fp32 for the last 3 tiles
          (the rest dump to a scratch tile); s = Sign(W) fp8 on ACT for all
          16 tiles.  Ternary = sign * mask needs |W| in FP32: a bf16 |W|
          flips ~1e-3 of the weights at the threshold (4e-2 output error).
  pass 2: thresholds 0.5*(gamma+eps) via ones-matmul partition reduce;
          ternarize is ONE DVE op per tile:
          wq = (|W| > thr) * s  (scalar_tensor_tensor IS_GT/MULT, fp8 out).
          Retained tiles 15..13 first, then re-stream W descending (12..0,
          |W| recomputed on ACT) so k-pairs complete earliest-first. The
          x-prep (DMA + ACT hi-cast + GPSIMD lo-sub) for the first epochs
          is hoisted before / interleaved into the re-stream so the casts
          are not stuck behind the re-stream Abs ops on the ACT queue.
  main:   epochs of 2 token-tiles x 4 output chunks = 8 concurrent
          [128,512] PSUM groups; 12 DoubleRow matmuls per group in
          quantize-completion order; bias-add fused into the PSUM eviction
          on DVE; fp32 out. The final epoch emits group-major so evictions
          and out-DMA overlap the last matmuls.
"""

from contextlib import ExitStack

import numpy as np

import concourse.bacc as bacc
import concourse.bass as bass
import concourse.mybir as mybir
import concourse.tile as tile
from concourse.bass_utils import run_bass_kernel_spmd

P = 128
B, S, D_IN, D_OUT = 8, 4096, 2048, 2048
N_CORES = 8
TOK = (B * S) // N_CORES          # 4096 tokens per core
KT = D_IN // P                    # 16 k-tiles
KK = KT // 2                      # 8 k-pairs (DoubleRow contracts 2 tiles)
L = 4                             # k-pairs receiving the lo correction
TT = TOK // P                     # 32 token tiles
NC_CHUNK = 512                    # matmul moving free dim (1 PSUM bank fp32)
OC = D_OUT // NC_CHUNK            # 4 output chunks
W_ELEMS = D_OUT * D_IN            # 2**22 (power of 2: S/N == S*(1/N) exactly)
EPS = 1e-5
CKP0 = KK - L                     # first corrected k-pair (tiles 8..15)
NRET_A = 3                        # |W| tiles retained across the threshold
ACT_P1 = (13, 14, 15, 1, 5, 9)    # pass-1 tiles reduced on ACT (rest DVE)

F32 = mybir.dt.float32
BF16 = mybir.dt.bfloat16
FP8 = mybir.dt.float8e4
DR = mybir.MatmulPerfMode.DoubleRow
MULT = mybir.AluOpType.mult
ADD = mybir.AluOpType.add
IS_GT = mybir.AluOpType.is_gt
AX_X = mybir.AxisListType.X


def build_nc() -> bass.Bass:
    nc = bacc.Bacc(None, target_bir_lowering=False)
    xT = nc.dram_tensor("xT", [D_IN, TOK], F32, kind="ExternalInput")
    WT = nc.dram_tensor("WT", [D_IN, D_OUT], F32, kind="ExternalInput")
    b = nc.dram_tensor("b", [D_OUT], F32, kind="ExternalInput")
    out = nc.dram_tensor("out", [TOK, D_OUT], F32, kind="ExternalOutput")

    with tile.TileContext(nc) as tc, ExitStack() as ctx:
        wpool = ctx.enter_context(tc.tile_pool(name="wpass", bufs=3))
        awpool = ctx.enter_context(tc.tile_pool(name="aw", bufs=NRET_A + 1))
        spool = ctx.enter_context(tc.tile_pool(name="scalars", bufs=1))
        sgnpool = ctx.enter_context(tc.tile_pool(name="sgn", bufs=1))
        wqpool = ctx.enter_context(tc.tile_pool(name="wq", bufs=1))
        xfpool = ctx.enter_context(tc.tile_pool(name="xf", bufs=4))
        xhpool = ctx.enter_context(tc.tile_pool(name="xh", bufs=6))
        xlpool = ctx.enter_context(tc.tile_pool(name="xl", bufs=6))
        opool = ctx.enter_context(tc.tile_pool(name="osb", bufs=2))
        pspool = ctx.enter_context(
            tc.tile_pool(name="psum", bufs=8, space="PSUM")
        )

        xT_v = xT.rearrange("(a p) t -> p a t", p=P)  # [128, KT, TOK]
        xhs, xls = {}, {}
        first_xf_dma = [True]

        def emit_xprep(tt):
            xf = xfpool.tile([P, KT, P], F32, tag="xf")
            xf_dma = nc.gpsimd.dma_start(
                xf[:], xT_v[:, :, tt * P : (tt + 1) * P]
            )
            if first_xf_dma[0]:
                # x competes with the gamma-critical W stream for HBM;
                # hold it back until pass 1 is issued.
                first_xf_dma[0] = False
                tile.add_dep_helper(
                    xf_dma.ins, last_w1_dma.ins, reason="defer x behind pass1"
                )
            xh = xhpool.tile([P, KT, P], FP8, tag="xh")
            nc.scalar.activation(
                xh[:], xf[:], mybir.ActivationFunctionType.Copy
            )
            xl = xlpool.tile([P, 2 * L, P], FP8, tag="xl")
            nc.gpsimd.tensor_sub(
                xl[:], xf[:, 2 * CKP0 :, :], xh[:, 2 * CKP0 :, :]
            )
            xhs[tt], xls[tt] = xh, xl

        # ---- pass 1: stream W; gamma partials split DVE/ACT; s = Sign(W)
        # on ACT (all tiles); |W| kept fp32 for tiles 15..13.
        partials_dve = spool.tile([P, KT - len(ACT_P1)], F32)
        partials_act = spool.tile([P, len(ACT_P1)], F32)
        dump = spool.tile([P, D_OUT], BF16)
        sgn = sgnpool.tile([P, KT, D_OUT], FP8)
        aw_resident = {}
        last_w1_dma = None
        idv = iac = 0
        for kt in range(KT):
            wt = wpool.tile([P, D_OUT], F32, tag="wt", name=f"w1_{kt}")
            last_w1_dma = nc.sync.dma_start(wt[:], WT[kt * P : (kt + 1) * P, :])
            if kt in ACT_P1:
                if kt >= KT - NRET_A:
                    ao = awpool.tile([P, D_OUT], F32, tag="aw", name=f"aw{kt}")
                    aw_resident[kt] = ao
                    ao_ap = ao[:]
                else:
                    ao_ap = dump[:]
                nc.scalar.activation(
                    ao_ap,
                    wt[:],
                    mybir.ActivationFunctionType.Abs,
                    accum_out=partials_act[:, iac : iac + 1],
                )
                iac += 1
            else:
                nc.vector.reduce_sum(
                    partials_dve[:, idv : idv + 1],
                    wt[:],
                    axis=AX_X,
                    apply_absolute_value=True,
                )
                idv += 1
            nc.scalar.sign(sgn[:, kt, :], wt[:])

        c1 = spool.tile([P, 1], F32)
        nc.vector.reduce_sum(c1[:], partials_dve[:], axis=AX_X)
        c2 = spool.tile([P, 1], F32)
        nc.vector.reduce_sum(c2[:], partials_act[:], axis=AX_X)
        colsum = spool.tile([P, 1], F32)
        nc.vector.tensor_add(colsum[:], c1[:], c2[:])

        # Partition reduce + broadcast in one PE op.
        ones_sq = spool.tile([P, P], F32)
        nc.vector.memset(ones_sq[:], 1.0)
        total_ps = pspool.tile([P, NC_CHUNK], F32, tag="ps")
        nc.tensor.matmul(
            total_ps[:, 0:1], ones_sq[:], colsum[:], start=True, stop=True
        )

        # threshold: Wq nonzero  <=>  |W| > 0.5*(gamma+eps)
        geps = spool.tile([P, 1], F32)
        nc.vector.tensor_scalar(
            geps[:], total_ps[:, 0:1], 1.0 / W_ELEMS, EPS, MULT, ADD
        )
        thr = spool.tile([P, 1], F32)
        nc.vector.tensor_scalar_mul(thr[:], geps[:], 0.5)

        # Bias replicated to all partitions (gpsimd queue; sync stays on W).
        bias_sb = spool.tile([P, D_OUT], F32)
        b_row = b[:].rearrange("(o d) -> o d", o=1)
        bias_dma = nc.gpsimd.dma_start(bias_sb[:], b_row.to_broadcast((P, D_OUT)))
        tile.add_dep_helper(
            bias_dma.ins, last_w1_dma.ins, reason="defer bias behind pass1"
        )

        # x-prep for epochs 0-1 before the re-stream Abs ops hit the ACT
        # queue (casts must not wait behind them).
        for tt in range(4):
            emit_xprep(tt)

        # ---- pass 2: wq = (|W| > thr) * s, one DVE op per tile ----
        wq8 = wqpool.tile([P, KT, D_OUT], FP8)

        def emit_quant(kt):
            if kt in aw_resident:
                aw_ap = aw_resident[kt][:]
            else:
                wt = wpool.tile([P, D_OUT], F32, tag="wt", name=f"w2_{kt}")
                nc.sync.dma_start(wt[:], WT[kt * P : (kt + 1) * P, :])
                aw = awpool.tile([P, D_OUT], F32, tag="aw", name=f"aw2_{kt}")
                nc.scalar.activation(
                    aw[:], wt[:], mybir.ActivationFunctionType.Abs
                )
                aw_ap = aw[:]
            nc.vector.scalar_tensor_tensor(
                wq8[:, kt, :], aw_ap, thr[:], sgn[:, kt, :], IS_GT, MULT
            )

        for kt in (15, 14, 13, 12, 11, 10, 9, 8, 7, 6):
            emit_quant(kt)
        # ep2's x-prep lands between re-stream chunks so its ACT cast runs
        # before the tail Abs ops.
        for tt in (4, 5):
            emit_xprep(tt)
        for kt in (5, 4, 3, 2, 1, 0):
            emit_quant(kt)

        # Per-group matmul emission order (matches quantize completion):
        # pairs 7,6 first (resident + first re-streams), lo follows its hi
        # pair, tail pairs 3..0 last.
        MM_ORDER = (
            [("h", 7), ("l", 7), ("h", 6), ("l", 6)]
            + [("h", 5), ("l", 5), ("h", 4), ("l", 4)]
            + [("h", 3), ("h", 2), ("h", 1), ("h", 0)]
        )

        # ---- main: out[t, :] = x[t, :] @ WqT + b ----
        TPE = 2  # token tiles per epoch
        NEP = TT // TPE
        for ep in range(NEP):
            for i in range(TPE):
                tt = ep * TPE + i
                if tt not in xhs:
                    emit_xprep(tt)

            groups = [(i, oc) for i in range(TPE) for oc in range(OC)]
            pss = [
                pspool.tile([P, NC_CHUNK], F32, tag="ps", name=f"ps{g}")
                for g in range(len(groups))
            ]

            def emit_mm(g, mi):
                i, oc = groups[g]
                kind, kkp = MM_ORDER[mi]
                tt = ep * TPE + i
                if kind == "h":
                    stat = xhs[tt][:, 2 * kkp : 2 * kkp + 2, :]
                else:
                    stat = xls[tt][:, 2 * (kkp - CKP0) : 2 * (kkp - CKP0) + 2, :]
                nc.tensor.matmul(
                    pss[g][:],
                    stat,
                    wq8[:, 2 * kkp : 2 * kkp + 2,
                        oc * NC_CHUNK : (oc + 1) * NC_CHUNK],
                    start=(mi == 0),
                    stop=(mi == len(MM_ORDER) - 1),
                    perf_mode=DR,
                )

            if ep < NEP - 1:
                for mi in range(len(MM_ORDER)):
                    for g in range(len(groups)):
                        emit_mm(g, mi)
            else:
                # final epoch: group-major so early groups' evictions and
                # out-DMA overlap the remaining groups' matmuls.
                for g in range(len(groups)):
                    for mi in range(len(MM_ORDER)):
                        emit_mm(g, mi)

            for i in range(TPE):
                tt = ep * TPE + i
                osb = opool.tile([P, D_OUT], F32, tag="osb")
                for oc in range(OC):
                    nc.vector.tensor_add(
                        osb[:, oc * NC_CHUNK : (oc + 1) * NC_CHUNK],
                        pss[i * OC + oc][:],
                        bias_sb[:, oc * NC_CHUNK : (oc + 1) * NC_CHUNK],
                    )
                nc.sync.dma_start(out[tt * P : (tt + 1) * P, :], osb[:])
                del xhs[tt], xls[tt]

    nc.finalize()
    return nc


_NC_CACHE: list = []


def _get_nc() -> bass.Bass:
    if not _NC_CACHE:
        _NC_CACHE.append(build_nc())
    return _NC_CACHE[0]


def make_in_maps(x: np.ndarray, W: np.ndarray, b: np.ndarray):
    x = np.asarray(x, dtype=np.float32).reshape(N_CORES, TOK, D_IN)
    W = np.asarray(W, dtype=np.float32)
    b = np.asarray(b, dtype=np.float32)
    WT = np.ascontiguousarray(W.T)
    return [
        {"xT": np.ascontiguousarray(x[c].T), "WT": WT, "b": b}
        for c in range(N_CORES)
    ]


def run(x, W, b, **spmd_kwargs):
    """Run the SPMD kernel; returns (full_output, BassKernelResults)."""
    nc = _get_nc()
    in_maps = make_in_maps(x, W, b)
    res = run_bass_kernel_spmd(nc, in_maps, list(range(N_CORES)), **spmd_kwargs)
    out = np.stack([res.results[c]["out"] for c in range(N_CORES)], axis=0)
    return out.reshape(B, S, D_OUT), res


def kernel(x, W, b):
    out, _ = run(x, W, b)
    return out


# revision 11
# speedup vs baseline: 1.0199x; 1.0199x over previous
"""BitLinear158 Trainium2 kernel — fp8 DoubleRow with partial hi/lo correction.

Reference computation:
    gamma = mean(|W|)
    Wq    = clip(round(W / (gamma + 1e-5)), -1, 1)      # ternary {-1, 0, +1}
    out   = x @ Wq.T + b                                # x: [8, 4096, 2048]

Sharding: data-parallel over the batch dim (8 batches -> 8 cores). Each core
gets x[i] (host-transposed to k-major), the full W (host-transposed) and b.
gamma is computed redundantly per-core -- measured cross-core collective
latency/skew (~80us) far exceeds the 45us it would save.

Math: Wq is ternary so it is EXACT in fp8e4 (e4m3). The fp8 DoubleRow matmul
contracts K=256 per instruction at the same per-instruction cost as a bf16
K=128 matmul (measured ~216ns at 512 free rows) -> 2x FLOP rate. Activations
split x = hi + lo with hi = fp8(x), lo = fp8(x - hi); hi covers all 16
k-tiles, lo corrects k-tiles 8..15 (L=4 of 8 k-pairs). Output L2 rel error
= 2.35e-2 * sqrt(1 - L/8) -> 1.66e-2 measured on HW (gate 2e-2).

Device pipeline per core:
  pass 1: stream WT (16 MiB, DMA-bound ~50us). During the stream: gamma
          partials via DVE reduce_sum(|.|) on 10 tiles and ACT Abs+accum on
          6; the ACT path's |W| output is KEPT in fp32 for the last 3 tiles
          (the rest dump to a scratch tile); s = Sign(W) fp8 on ACT for all
          16 tiles.  Ternary = sign * mask needs |W| in FP32: a bf16 |W|
          flips ~1e-3 of the weights at the threshold (4e-2 output error).
  pass 2: thresholds 0.5*(gamma+eps) via ones-matmul partition reduce;
          ternarize is ONE DVE op per tile:
          wq = (|W| > thr) * s  (scalar_tensor_tensor IS_GT/MULT, fp8 out).
          Retained tiles 15..13 first, then re-stream W descending (12..0,
          |W| recomputed on ACT) so k-pairs complete earliest-first. The
          x-prep (DMA + ACT hi-cast + GPSIMD lo-sub) for the first epochs
          is hoisted before / interleaved into the re-stream so the casts
          are not stuck behind the re-stream Abs ops on the ACT queue.
  main:   epochs of 2 token-tiles x 4 output chunks = 8 concurrent
          [128,512] PSUM groups; 12 DoubleRow matmuls per group in
          quantize-completion order; bias-add fused into the PSUM eviction
          on DVE; fp32 out. The final epoch emits group-major so evictions
          and out-DMA overlap the last matmuls.
"""

from contextlib import ExitStack

import numpy as np

import concourse.bacc as bacc
import concourse.bass as bass
import concourse.mybir as mybir
import concourse.tile as tile
from concourse.bass_utils import run_bass_kernel_spmd

P = 128
B, S, D_IN, D_OUT = 8, 4096, 2048, 2048
N_CORES = 8
TOK = (B * S) // N_CORES          # 4096 tokens per core
KT = D_IN // P                    # 16 k-tiles
KK = KT // 2                      # 8 k-pairs (DoubleRow contracts 2 tiles)
L = 4                             # k-pairs receiving the lo correction
TT = TOK // P                     # 32 token tiles
NC_CHUNK = 512                    # matmul moving free dim (1 PSUM bank fp32)
OC = D_OUT // NC_CHUNK            # 4 output chunks
W_ELEMS = D_OUT * D_IN            # 2**22 (power of 2: S/N == S*(1/N) exactly)
EPS = 1e-5
CKP0 = KK - L                     # first corrected k-pair (tiles 8..15)
NRET_A = 2                        # |W| tiles retained across the threshold
ACT_P1 = (14, 15, 1, 5, 9)        # pass-1 tiles reduced on ACT (rest DVE)

F32 = mybir.dt.float32
BF16 = mybir.dt.bfloat16
FP8 = mybir.dt.float8e4
DR = mybir.MatmulPerfMode.DoubleRow
MULT = mybir.AluOpType.mult
ADD = mybir.AluOpType.add
IS_GT = mybir.AluOpType.is_gt
AX_X = mybir.AxisListType.X


def build_nc() -> bass.Bass:
    nc = bacc.Bacc(None, target_bir_lowering=False)
    xT = nc.dram_tensor("xT", [D_IN, TOK], F32, kind="ExternalInput")
    WT = nc.dram_tensor("WT", [D_IN, D_OUT], F32, kind="ExternalInput")
    b = nc.dram_tensor("b", [D_OUT], F32, kind="ExternalInput")
    out = nc.dram_tensor("out", [TOK, D_OUT], F32, kind="ExternalOutput")

    with tile.TileContext(nc) as tc, ExitStack() as ctx:
        wpool = ctx.enter_context(tc.tile_pool(name="wpass", bufs=4))
        awpool = ctx.enter_context(tc.tile_pool(name="aw", bufs=NRET_A + 1))
        spool = ctx.enter_context(tc.tile_pool(name="scalars", bufs=1))
        sgnpool = ctx.enter_context(tc.tile_pool(name="sgn", bufs=1))
        wqpool = ctx.enter_context(tc.tile_pool(name="wq", bufs=1))
        xfpool = ctx.enter_context(tc.tile_pool(name="xf", bufs=4))
        xhpool = ctx.enter_context(tc.tile_pool(name="xh", bufs=6))
        xlpool = ctx.enter_context(tc.tile_pool(name="xl", bufs=6))
        opool = ctx.enter_context(tc.tile_pool(name="osb", bufs=2))
        pspool = ctx.enter_context(
            tc.tile_pool(name="psum", bufs=8, space="PSUM")
        )

        xT_v = xT.rearrange("(a p) t -> p a t", p=P)  # [128, KT, TOK]
        xhs, xls = {}, {}
        first_xf_dma = [True]

        def emit_xprep(tt):
            xf = xfpool.tile([P, KT, P], F32, tag="xf")
            xf_dma = nc.gpsimd.dma_start(
                xf[:], xT_v[:, :, tt * P : (tt + 1) * P]
            )
            if first_xf_dma[0]:
                # x competes with the gamma-critical W stream for HBM;
                # hold it back until pass 1 is issued.
                first_xf_dma[0] = False
                tile.add_dep_helper(
                    xf_dma.ins, last_w1_dma.ins, reason="defer x behind pass1"
                )
            xh = xhpool.tile([P, KT, P], FP8, tag="xh")
            nc.scalar.activation(
                xh[:], xf[:], mybir.ActivationFunctionType.Copy
            )
            xl = xlpool.tile([P, 2 * L, P], FP8, tag="xl")
            nc.gpsimd.tensor_sub(
                xl[:], xf[:, 2 * CKP0 :, :], xh[:, 2 * CKP0 :, :]
            )
            xhs[tt], xls[tt] = xh, xl

        # ---- pass 1: stream W; gamma partials split DVE/ACT; s = Sign(W)
        # on ACT (all tiles); |W| kept fp32 for tiles 15..13.
        partials_dve = spool.tile([P, KT - len(ACT_P1)], F32)
        partials_act = spool.tile([P, len(ACT_P1)], F32)
        dump = spool.tile([P, D_OUT], BF16)
        sgn = sgnpool.tile([P, KT, D_OUT], FP8)
        aw_resident = {}
        last_w1_dma = None
        idv = iac = 0
        for kt in range(KT):
            wt = wpool.tile([P, D_OUT], F32, tag="wt", name=f"w1_{kt}")
            last_w1_dma = nc.sync.dma_start(wt[:], WT[kt * P : (kt + 1) * P, :])
            if kt in ACT_P1:
                if kt >= KT - NRET_A:
                    ao = awpool.tile([P, D_OUT], F32, tag="aw", name=f"aw{kt}")
                    aw_resident[kt] = ao
                    ao_ap = ao[:]
                else:
                    ao_ap = dump[:]
                nc.scalar.activation(
                    ao_ap,
                    wt[:],
                    mybir.ActivationFunctionType.Abs,
                    accum_out=partials_act[:, iac : iac + 1],
                )
                iac += 1
            else:
                nc.vector.reduce_sum(
                    partials_dve[:, idv : idv + 1],
                    wt[:],
                    axis=AX_X,
                    apply_absolute_value=True,
                )
                idv += 1
            nc.scalar.sign(sgn[:, kt, :], wt[:])

        c1 = spool.tile([P, 1], F32)
        nc.vector.reduce_sum(c1[:], partials_dve[:], axis=AX_X)
        c2 = spool.tile([P, 1], F32)
        nc.vector.reduce_sum(c2[:], partials_act[:], axis=AX_X)
        colsum = spool.tile([P, 1], F32)
        nc.vector.tensor_add(colsum[:], c1[:], c2[:])

        # Partition reduce + broadcast in one PE op.
        ones_sq = spool.tile([P, P], F32)
        nc.vector.memset(ones_sq[:], 1.0)
        total_ps = pspool.tile([P, NC_CHUNK], F32, tag="ps")
        nc.tensor.matmul(
            total_ps[:, 0:1], ones_sq[:], colsum[:], start=True, stop=True
        )

        # threshold: Wq nonzero  <=>  |W| > 0.5*(gamma+eps)
        geps = spool.tile([P, 1], F32)
        nc.vector.tensor_scalar(
            geps[:], total_ps[:, 0:1], 1.0 / W_ELEMS, EPS, MULT, ADD
        )
        thr = spool.tile([P, 1], F32)
        nc.vector.tensor_scalar_mul(thr[:], geps[:], 0.5)

        bias_sb = spool.tile([P, D_OUT], F32)
        b_row = b[:].rearrange("(o d) -> o d", o=1)

        # x-prep for epochs 0-1 before the re-stream Abs ops hit the ACT
        # queue (casts must not wait behind them).
        for tt in range(4):
            emit_xprep(tt)

        # ---- pass 2: wq = (|W| > thr) * s, one DVE op per tile ----
        wq8 = wqpool.tile([P, KT, D_OUT], FP8)

        def emit_quant(kt):
            if kt in aw_resident:
                aw_ap = aw_resident[kt][:]
            else:
                wt = wpool.tile([P, D_OUT], F32, tag="wt", name=f"w2_{kt}")
                nc.sync.dma_start(wt[:], WT[kt * P : (kt + 1) * P, :])
                aw = awpool.tile([P, D_OUT], F32, tag="aw", name=f"aw2_{kt}")
                nc.scalar.activation(
                    aw[:], wt[:], mybir.ActivationFunctionType.Abs
                )
                aw_ap = aw[:]
            nc.vector.scalar_tensor_tensor(
                wq8[:, kt, :], aw_ap, thr[:], sgn[:, kt, :], IS_GT, MULT
            )

        for kt in (15, 14, 13, 12, 11, 10, 9, 8, 7, 6):
            emit_quant(kt)
        # Bias replicated to all partitions: emitted here so the sync-queue
        # broadcast neither delays pass 1 nor the early re-stream, and the
        # HWDGE does the 128-way replication (SWDGE is ~20us for this).
        nc.sync.dma_start(bias_sb[:], b_row.to_broadcast((P, D_OUT)))
        # ep2's x-prep lands between re-stream chunks so its ACT cast runs
        # before the tail Abs ops.
        for tt in (4, 5):
            emit_xprep(tt)
        for kt in (5, 4, 3, 2, 1, 0):
            emit_quant(kt)

        # Per-group matmul emission order (matches quantize completion):
        # pairs 7,6 first (resident + first re-streams), lo follows its hi
        # pair, tail pairs 3..0 last.
        MM_ORDER = (
            [("h", 7), ("l", 7), ("h", 6), ("l", 6)]
            + [("h", 5), ("l", 5), ("h", 4), ("l", 4)]
            + [("h", 3), ("h", 2), ("h", 1), ("h", 0)]
        )

        # ---- main: out[t, :] = x[t, :] @ WqT + b ----
        TPE = 2  # token tiles per epoch
        NEP = TT // TPE
        for ep in range(NEP):
            for i in range(TPE):
                tt = ep * TPE + i
                if tt not in xhs:
                    emit_xprep(tt)

            groups = [(i, oc) for i in range(TPE) for oc in range(OC)]
            pss = [
                pspool.tile([P, NC_CHUNK], F32, tag="ps", name=f"ps{g}")
                for g in range(len(groups))
            ]

            def emit_mm(g, mi):
                i, oc = groups[g]
                kind, kkp = MM_ORDER[mi]
                tt = ep * TPE + i
                if kind == "h":
                    stat = xhs[tt][:, 2 * kkp : 2 * kkp + 2, :]
                else:
                    stat = xls[tt][:, 2 * (kkp - CKP0) : 2 * (kkp - CKP0) + 2, :]
                nc.tensor.matmul(
                    pss[g][:],
                    stat,
                    wq8[:, 2 * kkp : 2 * kkp + 2,
                        oc * NC_CHUNK : (oc + 1) * NC_CHUNK],
                    start=(mi == 0),
                    stop=(mi == len(MM_ORDER) - 1),
                    perf_mode=DR,
                )

            if ep < NEP - 1:
                for mi in range(len(MM_ORDER)):
                    for g in range(len(groups)):
                        emit_mm(g, mi)
            else:
                # final epoch: group-major so early groups' evictions and
                # out-DMA overlap the remaining groups' matmuls.
                for g in range(len(groups)):
                    for mi in range(len(MM_ORDER)):
                        emit_mm(g, mi)

            for i in range(TPE):
                tt = ep * TPE + i
                osb = opool.tile([P, D_OUT], F32, tag="osb")
                for oc in range(OC):
                    nc.vector.tensor_add(
                        osb[:, oc * NC_CHUNK : (oc + 1) * NC_CHUNK],
                        pss[i * OC + oc][:],
                        bias_sb[:, oc * NC_CHUNK : (oc + 1) * NC_CHUNK],
                    )
                nc.sync.dma_start(out[tt * P : (tt + 1) * P, :], osb[:])
                del xhs[tt], xls[tt]

    nc.finalize()
    return nc


_NC_CACHE: list = []


def _get_nc() -> bass.Bass:
    if not _NC_CACHE:
        _NC_CACHE.append(build_nc())
    return _NC_CACHE[0]


def make_in_maps(x: np.ndarray, W: np.ndarray, b: np.ndarray):
    x = np.asarray(x, dtype=np.float32).reshape(N_CORES, TOK, D_IN)
    W = np.asarray(W, dtype=np.float32)
    b = np.asarray(b, dtype=np.float32)
    WT = np.ascontiguousarray(W.T)
    return [
        {"xT": np.ascontiguousarray(x[c].T), "WT": WT, "b": b}
        for c in range(N_CORES)
    ]


def run(x, W, b, **spmd_kwargs):
    """Run the SPMD kernel; returns (full_output, BassKernelResults)."""
    nc = _get_nc()
    in_maps = make_in_maps(x, W, b)
    res = run_bass_kernel_spmd(nc, in_maps, list(range(N_CORES)), **spmd_kwargs)
    out = np.stack([res.results[c]["out"] for c in range(N_CORES)], axis=0)
    return out.reshape(B, S, D_OUT), res


def kernel(x, W, b):
    out, _ = run(x, W, b)
    return out
